# revision 33
# baseline (speedup 1.0000x reference)
"""Trainium2 Bass kernel for a 2-layer GCN (PyG GCNConv semantics).

    out = Ahat @ relu(Ahat @ (X W1) + b1) @ W2 + b2,  Ahat = D^-1/2 (A+I) D^-1/2

Math restructure: norm(e) = dinv[src]*dinv[dst] is separable AND aggregation
commutes with the dense projections, so layer 1 aggregates rows of
xs = dinv ⊙ X directly: agg[d] = Σ xs[src], then h[d] = relu(dinv[d]·
(agg[d] @ W1) + b1) and u2[d] = dinv[d]·(h[d] @ W2) are computed densely per
128-dest window in transposed (feature-major) layout. Layer 2 aggregates u2
rows the same way.

Distribution (8 NeuronCores, SPMD): edges are partitioned by destination
(sharding-hint's edge-parallel scheme) — destinations dealt round-robin over
degree-sorted order so all cores share one compiled per-window round
schedule. The host stages each core's slot stream (source rows in
ELL/round order, one 128-lane slab per round — the "shard inputs" step of
the full-IO contract), so the device consumes plain contiguous streaming
DMAs (16KB descriptors on HWDGE) and TensorE identity-matmul accumulation.
Random-access row gathers on-device were 5-8ns/descriptor on the GPSIMD
SWDGE path (measured) — descriptor generation, not HBM bandwidth, bound;
streaming sidesteps descriptor generation entirely. Two SPMD dispatches:
P1 emits u2 shards (bf16); the host re-stages them edge-ordered (pure data
movement) and P2 aggregates layer 2.
"""

from contextlib import ExitStack

import ml_dtypes
import numpy as np

N, E, IN, HID, OUT = 50000, 600000, 128, 128, 64
NCORE = 8
P = 128
DPC = 6272  # dests per core (49 windows * 128) >= ceil(N/NCORE)
NW = DPC // P  # 49
PADROW = N  # table row N = zeros (pad slots)
BF16 = ml_dtypes.bfloat16
CH = 64  # stream chunk (rounds) per DMA

_CACHE = {}


# ---------------------------------------------------------------- host prep


def _prep(edge_index):
    row = np.asarray(edge_index[0], dtype=np.int64)
    col = np.asarray(edge_index[1], dtype=np.int64)
    deg = np.bincount(col, minlength=N) + 1  # in-degree + self
    dinv = (1.0 / np.sqrt(deg.astype(np.float64))).astype(np.float32)

    # shard dests: degree-sorted, dealt round-robin so core profiles match
    order = np.argsort(-deg, kind="stable")
    dests = np.full((NCORE, DPC), -1, np.int64)
    for c in range(NCORE):
        mine = order[c::NCORE]
        dests[c, : len(mine)] = mine

    slots = np.zeros((NCORE, DPC), np.int64)
    v = dests >= 0
    slots[v] = deg[dests[v]]
    R = np.maximum(slots.reshape(NCORE, NW, P).max(axis=(0, 2)), 1).astype(np.int64)
    offs = np.concatenate([[0], np.cumsum(R)])
    TOT = int(offs[-1])

    # edges grouped by dest
    eorder = np.argsort(col, kind="stable")
    srcs_sorted = row[eorder]
    cnt = np.bincount(col, minlength=N)
    starts = np.concatenate([[0], np.cumsum(cnt)])[:N]

    R0 = int(R.max()) + 1
    idx_all = np.full((NCORE, P, TOT), PADROW, np.int32)
    dinv_win = np.zeros((NCORE, P, NW), np.float32)
    rr = np.arange(R0)[None, :]
    for c in range(NCORE):
        d = dests[c]  # [DPC]
        dc = np.clip(d, 0, N - 1)
        dcnt = np.where(d >= 0, deg[dc], 0)
        dstart = np.where(d >= 0, starts[dc], 0)
        gpos = np.clip(dstart[:, None] + rr - 1, 0, E - 1)
        ed = srcs_sorted[gpos]  # table row of edge source
        arr = np.where((rr >= 1) & (rr < dcnt[:, None]), ed, PADROW)
        arr[:, 0] = np.where(d >= 0, d, PADROW)  # self slot
        a3 = arr.astype(np.int32).reshape(NW, P, R0)
        for w in range(NW):
            idx_all[c, :, offs[w] : offs[w + 1]] = a3[w, :, : R[w]]
        dv = np.where(d >= 0, dinv[dc], 0.0).astype(np.float32)
        dinv_win[c] = dv.reshape(NW, P).T

    return {
        "dinv": dinv,
        "dests": dests,
        "R": tuple(int(r) for r in R),
        "offs": offs,
        "TOT": TOT,
        "idx_all": idx_all,
        "dinv_win": dinv_win,
    }


# ------------------------------------------------------------- bass builders


def _new_nc():
    import concourse.bacc as bacc

    return bacc.Bacc("TRN2", target_bir_lowering=False, debug=False, num_devices=NCORE)


def _stream(nc, gpool, src_ap, TOT, fdim, dt, nm):
    """Chunked contiguous stream accessor: fetch(col) -> (tile, offset)."""
    staged = {}

    def fetch(col):
        ci = col // CH
        t = staged.get(ci)
        if t is None:
            s = ci * CH
            sz = min(CH, TOT - s)
            t = gpool.tile([P, CH * fdim], dt, tag=f"st{nm}", name=f"st{nm}{ci}")
            nc.sync.dma_start(
                out=t[:, : sz * fdim], in_=src_ap[:, s * fdim : (s + sz) * fdim]
            )
            staged[ci] = t
        return t, col - ci * CH

    return fetch


def _build_p1(prep, nrep=None):
    import concourse.tile as tile
    from concourse import mybir
    from concourse.masks import make_identity

    nc = _new_nc()
    R, offs, TOT = prep["R"], prep["offs"], prep["TOT"]
    f32, bf16 = mybir.dt.float32, mybir.dt.bfloat16
    xst = nc.declare_dram_parameter("xst", [P, TOT * HID], bf16, isOutput=False)
    W1m = nc.declare_dram_parameter("W1m", [IN, HID], bf16, isOutput=False)
    W2m = nc.declare_dram_parameter("W2m", [HID, OUT], bf16, isOutput=False)
    b1c = nc.declare_dram_parameter("b1c", [P, 1], f32, isOutput=False)
    dinv_w = nc.declare_dram_parameter("dinv_w", [P, NW], f32, isOutput=False)
    u2sT = nc.declare_dram_parameter("u2sT", [OUT, DPC], bf16, isOutput=True)

    with tile.TileContext(nc) as tc, ExitStack() as ctx:
        cpool = ctx.enter_context(tc.tile_pool(name="const", bufs=1))
        gpool = ctx.enter_context(tc.tile_pool(name="gath", bufs=4))
        bpool = ctx.enter_context(tc.tile_pool(name="work", bufs=3))
        apool = ctx.enter_context(tc.tile_pool(name="acc", bufs=3, space="PSUM"))
        tpool = ctx.enter_context(tc.tile_pool(name="ptr", bufs=2, space="PSUM"))
        hpool = ctx.enter_context(tc.tile_pool(name="ph", bufs=2, space="PSUM"))
        upool = ctx.enter_context(tc.tile_pool(name="pu", bufs=1, space="PSUM"))

        identB = cpool.tile([P, P], bf16)
        make_identity(nc, identB[:])
        w1sb = cpool.tile([IN, HID], bf16)
        nc.sync.dma_start(out=w1sb[:], in_=W1m[:])
        w2sb = cpool.tile([HID, OUT], bf16)
        nc.sync.dma_start(out=w2sb[:], in_=W2m[:])
        b1sb = cpool.tile([P, 1], f32)
        nc.sync.dma_start(out=b1sb[:], in_=b1c[:])
        dw_sb = cpool.tile([P, NW], f32)
        nc.sync.dma_start(out=dw_sb[:], in_=dinv_w[:])

        rep = tc.For_i(0, nrep, 1) if nrep else None
        if rep is not None:
            rep.__enter__()

        fetch = _stream(nc, gpool, xst, TOT, HID, bf16, "x")

        for w in range(NW):
            rw = int(R[w])
            acc = apool.tile([P, HID], f32, space="PSUM")
            for r in range(rw):
                t, co = fetch(int(offs[w]) + r)
                nc.tensor.matmul(
                    out=acc[:], lhsT=identB[:],
                    rhs=t[:, co * HID : (co + 1) * HID],
                    start=(r == 0), stop=(r == rw - 1),
                )
            # agg (dest-major) scaled by dinv[d], cast bf16
            aggsb = bpool.tile([P, HID], bf16, tag="aggsb")
            nc.scalar.activation(
                out=aggsb[:], in_=acc[:],
                func=mybir.ActivationFunctionType.Copy, scale=dw_sb[:, w : w + 1],
            )
            # transpose -> feature-major aggT[k, d]
            psT = tpool.tile([P, P], bf16, space="PSUM")
            nc.tensor.transpose(out=psT[:], in_=aggsb[:], identity=identB[:])
            aggT = bpool.tile([P, P], bf16, tag="aggT")
            nc.vector.tensor_copy(aggT[:], psT[:])
            # hT = relu(W1^T @ aggT + b1)
            psH = hpool.tile([P, P], f32, space="PSUM")
            nc.tensor.matmul(out=psH[:], lhsT=w1sb[:], rhs=aggT[:], start=True, stop=True)
            hT = bpool.tile([P, P], bf16, tag="hT")
            nc.scalar.activation(
                out=hT[:], in_=psH[:],
                func=mybir.ActivationFunctionType.Relu, bias=b1sb[:, 0:1],
            )
            # u2T = W2^T @ hT (OUT x dests), emitted feature-major raw;
            # the host applies the second dinv[d] while re-staging for P2
            psU = upool.tile([OUT, P], f32, space="PSUM")
            nc.tensor.matmul(out=psU[:], lhsT=w2sb[:], rhs=hT[:], start=True, stop=True)
            u2T = bpool.tile([OUT, P], bf16, tag="u2T")
            nc.vector.tensor_copy(u2T[:], psU[:])
            nc.sync.dma_start(out=u2sT[:, w * P : (w + 1) * P], in_=u2T[:])

        if rep is not None:
            rep.__exit__(None, None, None)

    nc.compile()
    return nc


def _build_p2(prep, nrep=None):
    import concourse.tile as tile
    from concourse import mybir
    from concourse.masks import make_identity

    nc = _new_nc()
    R, offs, TOT = prep["R"], prep["offs"], prep["TOT"]
    f32, bf16 = mybir.dt.float32, mybir.dt.bfloat16
    xut = nc.declare_dram_parameter("xut", [P, TOT * OUT], bf16, isOutput=False)
    dinv_w = nc.declare_dram_parameter("dinv_w", [P, NW], f32, isOutput=False)
    b2t = nc.declare_dram_parameter("b2t", [P, OUT], f32, isOutput=False)
    outs = nc.declare_dram_parameter("outs", [DPC, OUT], f32, isOutput=True)

    with tile.TileContext(nc) as tc, ExitStack() as ctx:
        cpool = ctx.enter_context(tc.tile_pool(name="const", bufs=1))
        gpool = ctx.enter_context(tc.tile_pool(name="gath", bufs=4))
        bpool = ctx.enter_context(tc.tile_pool(name="work", bufs=3))
        qpool = ctx.enter_context(tc.tile_pool(name="psum", bufs=3, space="PSUM"))

        identB = cpool.tile([P, P], bf16)
        make_identity(nc, identB[:])
        dw_sb = cpool.tile([P, NW], f32)
        nc.sync.dma_start(out=dw_sb[:], in_=dinv_w[:])
        b2sb = cpool.tile([P, OUT], f32)
        nc.sync.dma_start(out=b2sb[:], in_=b2t[:])

        rep = tc.For_i(0, nrep, 1) if nrep else None
        if rep is not None:
            rep.__enter__()

        fetch = _stream(nc, gpool, xut, TOT, OUT, bf16, "u")

        for w in range(NW):
            rw = int(R[w])
            acc = qpool.tile([P, OUT], f32, space="PSUM")
            for r in range(rw):
                t, co = fetch(int(offs[w]) + r)
                nc.tensor.matmul(
                    out=acc[:], lhsT=identB[:],
                    rhs=t[:, co * OUT : (co + 1) * OUT],
                    start=(r == 0), stop=(r == rw - 1),
                )
            m1 = bpool.tile([P, OUT], f32, tag="m1")
            nc.scalar.activation(
                out=m1[:], in_=acc[:],
                func=mybir.ActivationFunctionType.Copy, scale=dw_sb[:, w : w + 1],
            )
            o = bpool.tile([P, OUT], f32, tag="o")
            nc.vector.tensor_add(o[:], m1[:], b2sb[:])
            nc.sync.dma_start(out=outs[w * P : (w + 1) * P, :], in_=o[:])

        if rep is not None:
            rep.__exit__(None, None, None)

    nc.compile()
    return nc


# ------------------------------------------------------------------- driver


def _builds(prep):
    key = (prep["R"],)
    if key not in _CACHE:
        _CACHE[key] = (_build_p1(prep), _build_p2(prep))
    return _CACHE[key]


def kernel(x, edge_index, W1, b1, W2, b2):
    from concourse.bass_utils import run_bass_kernel_spmd

    x = np.asarray(x, np.float32)
    W1 = np.asarray(W1, np.float32)
    b1 = np.asarray(b1, np.float32)
    W2 = np.asarray(W2, np.float32)
    b2 = np.asarray(b2, np.float32)

    prep = _prep(edge_index)
    nc1, nc2 = _builds(prep)
    TOT = prep["TOT"]

    # source table: row s = dinv[s] * x[s], row N = zeros (pad slots)
    xs = np.zeros((N + 1, HID), BF16)
    xs[:N] = (prep["dinv"][:, None] * x).astype(BF16)
    b1c = np.broadcast_to(b1[:, None], (P, 1)).copy()
    b2t = np.broadcast_to(b2[None, :], (P, OUT)).copy()

    core_ids = list(range(NCORE))
    # edge-parallel input sharding: per-core ELL slot stream, lane-major
    in1 = [
        {
            "xst": xs[prep["idx_all"][c]].reshape(P, TOT * HID),
            "W1m": W1.astype(BF16),
            "W2m": W2.astype(BF16),
            "b1c": b1c,
            "dinv_w": prep["dinv_win"][c],
        }
        for c in core_ids
    ]
    res1 = run_bass_kernel_spmd(nc1, in1, core_ids)

    # host staging: scatter u2 shards into the node table (applying the
    # deferred dinv[d] scale), re-stage edge-ordered for layer 2
    u2tab = np.zeros((N + 1, OUT), BF16)
    for c in core_ids:
        d = prep["dests"][c]
        v = d >= 0
        raw = res1.results[c]["u2sT"].T[v].astype(np.float32)
        u2tab[d[v]] = (raw * prep["dinv"][d[v], None]).astype(BF16)

    in2 = [
        {
            "xut": u2tab[prep["idx_all"][c]].reshape(P, TOT * OUT),
            "dinv_w": prep["dinv_win"][c],
            "b2t": b2t,
        }
        for c in core_ids
    ]
    res2 = run_bass_kernel_spmd(nc2, in2, core_ids)

    out = np.zeros((N, OUT), np.float32)
    for c in core_ids:
        d = prep["dests"][c]
        v = d >= 0
        out[d[v]] = res2.results[c]["outs"][v]
    return out


# revision 35
# speedup vs baseline: 1.0280x; 1.0280x over previous
"""Trainium2 Bass kernel for a 2-layer GCN (PyG GCNConv semantics).

    out = Ahat @ relu(Ahat @ (X W1) + b1) @ W2 + b2,  Ahat = D^-1/2 (A+I) D^-1/2

Math restructure: norm(e) = dinv[src]*dinv[dst] is separable AND aggregation
commutes with the dense projections, so layer 1 aggregates rows of
xs = dinv ⊙ X directly: agg[d] = Σ xs[src], then h[d] = relu(dinv[d]·
(agg[d] @ W1) + b1) and u2[d] = dinv[d]·(h[d] @ W2) are computed densely per
128-dest window in transposed (feature-major) layout. Layer 2 aggregates u2
rows the same way.

Distribution (8 NeuronCores, SPMD): edges are partitioned by destination
(sharding-hint's edge-parallel scheme) — destinations dealt round-robin over
degree-sorted order so all cores share one compiled per-window round
schedule. The host stages each core's slot stream (source rows in
ELL/round order, one 128-lane slab per round — the "shard inputs" step of
the full-IO contract), so the device consumes plain contiguous streaming
DMAs (16KB descriptors on HWDGE) and TensorE identity-matmul accumulation.
Random-access row gathers on-device were 5-8ns/descriptor on the GPSIMD
SWDGE path (measured) — descriptor generation, not HBM bandwidth, bound;
streaming sidesteps descriptor generation entirely. Two SPMD dispatches:
P1 emits u2 shards (bf16); the host re-stages them edge-ordered (pure data
movement) and P2 aggregates layer 2.
"""

from contextlib import ExitStack

import ml_dtypes
import numpy as np

N, E, IN, HID, OUT = 50000, 600000, 128, 128, 64
NCORE = 8
P = 128
DPC = 6272  # dests per core (49 windows * 128) >= ceil(N/NCORE)
NW = DPC // P  # 49
PADROW = N  # table row N = zeros (pad slots)
BF16 = ml_dtypes.bfloat16
CH = 128  # stream chunk (rounds) per DMA

_CACHE = {}


# ---------------------------------------------------------------- host prep


def _prep(edge_index):
    row = np.asarray(edge_index[0], dtype=np.int64)
    col = np.asarray(edge_index[1], dtype=np.int64)
    deg = np.bincount(col, minlength=N) + 1  # in-degree + self
    dinv = (1.0 / np.sqrt(deg.astype(np.float64))).astype(np.float32)

    # shard dests: degree-sorted, dealt round-robin so core profiles match
    order = np.argsort(-deg, kind="stable")
    dests = np.full((NCORE, DPC), -1, np.int64)
    for c in range(NCORE):
        mine = order[c::NCORE]
        dests[c, : len(mine)] = mine

    slots = np.zeros((NCORE, DPC), np.int64)
    v = dests >= 0
    slots[v] = deg[dests[v]]
    R = np.maximum(slots.reshape(NCORE, NW, P).max(axis=(0, 2)), 1).astype(np.int64)
    offs = np.concatenate([[0], np.cumsum(R)])
    TOT = int(offs[-1])

    # edges grouped by dest
    eorder = np.argsort(col, kind="stable")
    srcs_sorted = row[eorder]
    cnt = np.bincount(col, minlength=N)
    starts = np.concatenate([[0], np.cumsum(cnt)])[:N]

    R0 = int(R.max()) + 1
    idx_all = np.full((NCORE, P, TOT), PADROW, np.int32)
    dinv_win = np.zeros((NCORE, P, NW), np.float32)
    rr = np.arange(R0)[None, :]
    for c in range(NCORE):
        d = dests[c]  # [DPC]
        dc = np.clip(d, 0, N - 1)
        dcnt = np.where(d >= 0, deg[dc], 0)
        dstart = np.where(d >= 0, starts[dc], 0)
        gpos = np.clip(dstart[:, None] + rr - 1, 0, E - 1)
        ed = srcs_sorted[gpos]  # table row of edge source
        arr = np.where((rr >= 1) & (rr < dcnt[:, None]), ed, PADROW)
        arr[:, 0] = np.where(d >= 0, d, PADROW)  # self slot
        a3 = arr.astype(np.int32).reshape(NW, P, R0)
        for w in range(NW):
            idx_all[c, :, offs[w] : offs[w + 1]] = a3[w, :, : R[w]]
        dv = np.where(d >= 0, dinv[dc], 0.0).astype(np.float32)
        dinv_win[c] = dv.reshape(NW, P).T

    return {
        "dinv": dinv,
        "dests": dests,
        "R": tuple(int(r) for r in R),
        "offs": offs,
        "TOT": TOT,
        "idx_all": idx_all,
        "dinv_win": dinv_win,
    }


# ------------------------------------------------------------- bass builders


def _new_nc():
    import concourse.bacc as bacc

    return bacc.Bacc("TRN2", target_bir_lowering=False, debug=False, num_devices=NCORE)


def _stream(nc, gpool, src_ap, TOT, fdim, dt, nm):
    """Chunked contiguous stream accessor: fetch(col) -> (tile, offset)."""
    staged = {}

    def fetch(col):
        ci = col // CH
        t = staged.get(ci)
        if t is None:
            s = ci * CH
            sz = min(CH, TOT - s)
            t = gpool.tile([P, CH * fdim], dt, tag=f"st{nm}", name=f"st{nm}{ci}")
            nc.sync.dma_start(
                out=t[:, : sz * fdim], in_=src_ap[:, s * fdim : (s + sz) * fdim]
            )
            staged[ci] = t
        return t, col - ci * CH

    return fetch


def _build_p1(prep, nrep=None):
    import concourse.tile as tile
    from concourse import mybir
    from concourse.masks import make_identity

    nc = _new_nc()
    R, offs, TOT = prep["R"], prep["offs"], prep["TOT"]
    f32, bf16 = mybir.dt.float32, mybir.dt.bfloat16
    xst = nc.declare_dram_parameter("xst", [P, TOT * HID], bf16, isOutput=False)
    W1m = nc.declare_dram_parameter("W1m", [IN, HID], bf16, isOutput=False)
    W2m = nc.declare_dram_parameter("W2m", [HID, OUT], bf16, isOutput=False)
    b1c = nc.declare_dram_parameter("b1c", [P, 1], f32, isOutput=False)
    dinv_w = nc.declare_dram_parameter("dinv_w", [P, NW], f32, isOutput=False)
    u2sT = nc.declare_dram_parameter("u2sT", [OUT, DPC], bf16, isOutput=True)

    with tile.TileContext(nc) as tc, ExitStack() as ctx:
        cpool = ctx.enter_context(tc.tile_pool(name="const", bufs=1))
        gpool = ctx.enter_context(tc.tile_pool(name="gath", bufs=4))
        bpool = ctx.enter_context(tc.tile_pool(name="work", bufs=3))
        apool = ctx.enter_context(tc.tile_pool(name="acc", bufs=3, space="PSUM"))
        tpool = ctx.enter_context(tc.tile_pool(name="ptr", bufs=2, space="PSUM"))
        hpool = ctx.enter_context(tc.tile_pool(name="ph", bufs=2, space="PSUM"))
        upool = ctx.enter_context(tc.tile_pool(name="pu", bufs=1, space="PSUM"))

        identB = cpool.tile([P, P], bf16)
        make_identity(nc, identB[:])
        w1sb = cpool.tile([IN, HID], bf16)
        nc.sync.dma_start(out=w1sb[:], in_=W1m[:])
        w2sb = cpool.tile([HID, OUT], bf16)
        nc.sync.dma_start(out=w2sb[:], in_=W2m[:])
        b1sb = cpool.tile([P, 1], f32)
        nc.sync.dma_start(out=b1sb[:], in_=b1c[:])
        dw_sb = cpool.tile([P, NW], f32)
        nc.sync.dma_start(out=dw_sb[:], in_=dinv_w[:])

        rep = tc.For_i(0, nrep, 1) if nrep else None
        if rep is not None:
            rep.__enter__()

        fetch = _stream(nc, gpool, xst, TOT, HID, bf16, "x")

        for w in range(NW):
            rw = int(R[w])
            acc = apool.tile([P, HID], f32, space="PSUM")
            for r in range(rw):
                t, co = fetch(int(offs[w]) + r)
                nc.tensor.matmul(
                    out=acc[:], lhsT=identB[:],
                    rhs=t[:, co * HID : (co + 1) * HID],
                    start=(r == 0), stop=(r == rw - 1),
                )
            # agg (dest-major) scaled by dinv[d], cast bf16
            aggsb = bpool.tile([P, HID], bf16, tag="aggsb")
            nc.scalar.activation(
                out=aggsb[:], in_=acc[:],
                func=mybir.ActivationFunctionType.Copy, scale=dw_sb[:, w : w + 1],
            )
            # transpose -> feature-major aggT[k, d]
            psT = tpool.tile([P, P], bf16, space="PSUM")
            nc.tensor.transpose(out=psT[:], in_=aggsb[:], identity=identB[:])
            aggT = bpool.tile([P, P], bf16, tag="aggT")
            nc.vector.tensor_copy(aggT[:], psT[:])
            # hT = relu(W1^T @ aggT + b1)
            psH = hpool.tile([P, P], f32, space="PSUM")
            nc.tensor.matmul(out=psH[:], lhsT=w1sb[:], rhs=aggT[:], start=True, stop=True)
            hT = bpool.tile([P, P], bf16, tag="hT")
            nc.scalar.activation(
                out=hT[:], in_=psH[:],
                func=mybir.ActivationFunctionType.Relu, bias=b1sb[:, 0:1],
            )
            # u2T = W2^T @ hT (OUT x dests), emitted feature-major raw;
            # the host applies the second dinv[d] while re-staging for P2
            psU = upool.tile([OUT, P], f32, space="PSUM")
            nc.tensor.matmul(out=psU[:], lhsT=w2sb[:], rhs=hT[:], start=True, stop=True)
            u2T = bpool.tile([OUT, P], bf16, tag="u2T")
            nc.vector.tensor_copy(u2T[:], psU[:])
            nc.sync.dma_start(out=u2sT[:, w * P : (w + 1) * P], in_=u2T[:])

        if rep is not None:
            rep.__exit__(None, None, None)

    nc.compile()
    return nc


def _build_p2(prep, nrep=None):
    import concourse.tile as tile
    from concourse import mybir
    from concourse.masks import make_identity

    nc = _new_nc()
    R, offs, TOT = prep["R"], prep["offs"], prep["TOT"]
    f32, bf16 = mybir.dt.float32, mybir.dt.bfloat16
    xut = nc.declare_dram_parameter("xut", [P, TOT * OUT], bf16, isOutput=False)
    dinv_w = nc.declare_dram_parameter("dinv_w", [P, NW], f32, isOutput=False)
    b2t = nc.declare_dram_parameter("b2t", [P, OUT], f32, isOutput=False)
    outs = nc.declare_dram_parameter("outs", [DPC, OUT], f32, isOutput=True)

    with tile.TileContext(nc) as tc, ExitStack() as ctx:
        cpool = ctx.enter_context(tc.tile_pool(name="const", bufs=1))
        gpool = ctx.enter_context(tc.tile_pool(name="gath", bufs=4))
        bpool = ctx.enter_context(tc.tile_pool(name="work", bufs=3))
        qpool = ctx.enter_context(tc.tile_pool(name="psum", bufs=6, space="PSUM"))

        identB = cpool.tile([P, P], bf16)
        make_identity(nc, identB[:])
        dw_sb = cpool.tile([P, NW], f32)
        nc.sync.dma_start(out=dw_sb[:], in_=dinv_w[:])
        b2sb = cpool.tile([P, OUT], f32)
        nc.sync.dma_start(out=b2sb[:], in_=b2t[:])

        rep = tc.For_i(0, nrep, 1) if nrep else None
        if rep is not None:
            rep.__enter__()

        fetch = _stream(nc, gpool, xut, TOT, OUT, bf16, "u")

        for w in range(NW):
            rw = int(R[w])
            acc = qpool.tile([P, OUT], f32, space="PSUM")
            for r in range(rw):
                t, co = fetch(int(offs[w]) + r)
                nc.tensor.matmul(
                    out=acc[:], lhsT=identB[:],
                    rhs=t[:, co * OUT : (co + 1) * OUT],
                    start=(r == 0), stop=(r == rw - 1),
                )
            m1 = bpool.tile([P, OUT], f32, tag="m1")
            nc.scalar.activation(
                out=m1[:], in_=acc[:],
                func=mybir.ActivationFunctionType.Copy, scale=dw_sb[:, w : w + 1],
            )
            o = bpool.tile([P, OUT], f32, tag="o")
            nc.vector.tensor_add(o[:], m1[:], b2sb[:])
            nc.sync.dma_start(out=outs[w * P : (w + 1) * P, :], in_=o[:])

        if rep is not None:
            rep.__exit__(None, None, None)

    nc.compile()
    return nc


# ------------------------------------------------------------------- driver


def _builds(prep):
    key = (prep["R"],)
    if key not in _CACHE:
        _CACHE[key] = (_build_p1(prep), _build_p2(prep))
    return _CACHE[key]


def kernel(x, edge_index, W1, b1, W2, b2):
    from concourse.bass_utils import run_bass_kernel_spmd

    x = np.asarray(x, np.float32)
    W1 = np.asarray(W1, np.float32)
    b1 = np.asarray(b1, np.float32)
    W2 = np.asarray(W2, np.float32)
    b2 = np.asarray(b2, np.float32)

    prep = _prep(edge_index)
    nc1, nc2 = _builds(prep)
    TOT = prep["TOT"]

    # source table: row s = dinv[s] * x[s], row N = zeros (pad slots)
    xs = np.zeros((N + 1, HID), BF16)
    xs[:N] = (prep["dinv"][:, None] * x).astype(BF16)
    b1c = np.broadcast_to(b1[:, None], (P, 1)).copy()
    b2t = np.broadcast_to(b2[None, :], (P, OUT)).copy()

    core_ids = list(range(NCORE))
    # edge-parallel input sharding: per-core ELL slot stream, lane-major
    in1 = [
        {
            "xst": xs[prep["idx_all"][c]].reshape(P, TOT * HID),
            "W1m": W1.astype(BF16),
            "W2m": W2.astype(BF16),
            "b1c": b1c,
            "dinv_w": prep["dinv_win"][c],
        }
        for c in core_ids
    ]
    res1 = run_bass_kernel_spmd(nc1, in1, core_ids)

    # host staging: scatter u2 shards into the node table (applying the
    # deferred dinv[d] scale), re-stage edge-ordered for layer 2
    u2tab = np.zeros((N + 1, OUT), BF16)
    for c in core_ids:
        d = prep["dests"][c]
        v = d >= 0
        raw = res1.results[c]["u2sT"].T[v].astype(np.float32)
        u2tab[d[v]] = (raw * prep["dinv"][d[v], None]).astype(BF16)

    in2 = [
        {
            "xut": u2tab[prep["idx_all"][c]].reshape(P, TOT * OUT),
            "dinv_w": prep["dinv_win"][c],
            "b2t": b2t,
        }
        for c in core_ids
    ]
    res2 = run_bass_kernel_spmd(nc2, in2, core_ids)

    out = np.zeros((N, OUT), np.float32)
    for c in core_ids:
        d = prep["dests"][c]
        v = d >= 0
        out[d[v]] = res2.results[c]["outs"][v]
    return out
